# revision 2
# baseline (speedup 1.0000x reference)
"""EquivariantDense kernel for Trainium2 (8 NeuronCores, data-parallel over batch).

Math: with K = 4096, N = 4K, shift = K, the reference computes
    out[b, i*O4 + o] = sum_j sum_k w_{j+1}[b, o, k] * x[b, ((i+j)%4)*K + k]
i.e. per batch, 4 weight matrices (1024, 4096) each hit the 4 chunks of x.

Device mapping (per core = one batch):
  - PE matmul out[m,n] = sum_p lhsT[p,m] * rhs[p,n] contracts over partitions,
    so weights are staged (on host) transposed to (k, o) layout.
  - stationary lhsT = x-chunk tile (128 k-part, 4 roll-columns) -> tiny LDWEIGHTS
  - moving rhs = W^T tile (128 k-part, 512 o)
  - accumulate all 128 k-blocks (4 j * 32 kb) into PSUM (4, 512) x 2 o-halves
  - weights are cast to bf16 on host: halves HBM traffic (memory-bound
    regime) and runs the PE at 1 cycle/row instead of fp32's 4. The output
    tolerance (2e-2) dwarfs bf16 quantization error (~2-3e-3).
"""

import numpy as np
import ml_dtypes

import concourse.mybir as mybir
import concourse.tile as tile
from concourse import bacc, bass_utils

B = 8
O4 = 1024
K = 4096
N = 4 * K  # 16384
NBLK = N // 128  # 128 global k-blocks of 128
KB2 = 4  # k-blocks per DMA tile
NT = NBLK // KB2  # 32 DMA tiles, 1 MiB each in bf16

BF16 = ml_dtypes.bfloat16

_nc_cache = None


def _build_program(repeat=1):
    # repeat>1 builds the same body repeated back-to-back; used only for
    # timing measurements (dispatch-overhead-free per-iteration estimates)
    nc = bacc.Bacc()
    f32 = mybir.dt.float32
    bf16 = mybir.dt.bfloat16
    xs_d = nc.dram_tensor("xstat", [128, NBLK * 4], bf16, kind="ExternalInput")
    wt_d = nc.dram_tensor("wt", [NT, 128, KB2 * O4], bf16, kind="ExternalInput")
    out_d = nc.dram_tensor("out", [4, O4], f32, kind="ExternalOutput")

    with tile.TileContext(nc) as tc:
        with (
            tc.tile_pool(name="xp", bufs=1) as xp,
            tc.tile_pool(name="wp", bufs=4) as wp,
            tc.tile_pool(name="pp", bufs=2, space="PSUM") as pp,
            tc.tile_pool(name="op", bufs=2) as op,
        ):
            xs = xp.tile([128, NBLK * 4], bf16)
            # SWDGE: keeps the SP HWDGE ring free for the weight stream.
            # (Loading xs via the ACT HWDGE ring instead correlated with
            # NRT_EXEC_UNIT_UNRECOVERABLE crashes under concurrent
            # dual-ring DMA; SWDGE here has been stable across many runs.)
            nc.gpsimd.dma_start(xs[:], xs_d[:])
            for _rep in range(repeat):
                ps0 = pp.tile([4, 512], f32, tag="ps0")
                ps1 = pp.tile([4, 512], f32, tag="ps1")
                # Read tiles highest-address-first: reverse of the input
                # upload order, so if the memory system keeps recently
                # written lines warm, the single cold pass hits them first.
                # Order is otherwise irrelevant (PSUM accumulation commutes).
                for tidx, t in enumerate(reversed(range(NT))):
                    w_tile = wp.tile([128, KB2 * O4], bf16, tag="w")
                    if tidx < NT - 1:
                        nc.sync.dma_start(w_tile[:], wt_d[t])
                    else:
                        # split the last-issued tile per k-block so the final
                        # matmuls chase the stream and the tail stays short;
                        # the final k-block splits again per o-half so the
                        # very last matmul waits on only 128 KiB
                        for kk in range(KB2 - 1):
                            nc.sync.dma_start(
                                w_tile[:, kk * O4 : (kk + 1) * O4],
                                wt_d[t, :, kk * O4 : (kk + 1) * O4],
                            )
                        kk = KB2 - 1
                        nc.sync.dma_start(
                            w_tile[:, kk * O4 : kk * O4 + 512],
                            wt_d[t, :, kk * O4 : kk * O4 + 512],
                        )
                        nc.sync.dma_start(
                            w_tile[:, kk * O4 + 512 : (kk + 1) * O4],
                            wt_d[t, :, kk * O4 + 512 : (kk + 1) * O4],
                        )
                    for kb2 in range(KB2):
                        g = t * KB2 + kb2
                        lhsT = xs[:, g * 4 : (g + 1) * 4]
                        first = tidx == 0 and kb2 == 0
                        last = tidx == NT - 1 and kb2 == KB2 - 1
                        nc.tensor.matmul(
                            ps0[:],
                            lhsT,
                            w_tile[:, kb2 * O4 : kb2 * O4 + 512],
                            start=first,
                            stop=last,
                        )
                        nc.tensor.matmul(
                            ps1[:],
                            lhsT,
                            w_tile[:, kb2 * O4 + 512 : (kb2 + 1) * O4],
                            start=first,
                            stop=last,
                        )
                ot = op.tile([4, O4], f32, tag="ot")
                nc.vector.tensor_copy(ot[:, 0:512], ps0[:])
                nc.scalar.copy(ot[:, 512:O4], ps1[:])
                nc.sync.dma_start(out_d[:], ot[:])
    nc.compile()
    return nc


def _get_program():
    global _nc_cache
    if _nc_cache is None:
        _nc_cache = _build_program()
    return _nc_cache


def prepare_inputs(x, w1, w2, w3, w4):
    """Host-side marshalling: shard over batch, cast bf16, transpose W to
    (k, o) tiles."""
    x = np.asarray(x, dtype=np.float32).astype(BF16)
    # Weight staging: W[b, j, o, k] -> Wh[b, t, p, kb2*O4 + o]
    # where k = (t*4 + kb2)*128 + p and j = (t*4 + kb2) // 32.
    W = np.stack(
        [np.asarray(w, dtype=np.float32).astype(BF16) for w in (w1, w2, w3, w4)],
        axis=1,
    )  # (B, 4, O4, K) bf16
    W6 = W.reshape(B, 4, O4, 8, KB2, 128)  # k = tq*512 + kb2*128 + p
    Wh = np.ascontiguousarray(W6.transpose(0, 1, 3, 5, 4, 2)).reshape(
        B, NT, 128, KB2 * O4
    )

    # x staging: xs[b, p, g*4 + c] = x[b, ((c + g//32) % 4)*K + (g%32)*128 + p]
    cols = np.arange(NBLK * 4)
    g = cols // 4
    c = cols % 4
    j = g // 32
    kb = g % 32
    src_base = ((c + j) % 4) * K + kb * 128  # (512,)
    xs = x[:, src_base[None, :] + np.arange(128)[:, None]]  # (B, 128, 512)
    xs = np.ascontiguousarray(xs)
    return xs, Wh


def run(x, w1, w2, w3, w4, trace=False, **kwargs):
    xs, Wh = prepare_inputs(x, w1, w2, w3, w4)
    nc = _get_program()
    in_maps = [{"xstat": xs[b], "wt": Wh[b]} for b in range(B)]
    res = bass_utils.run_bass_kernel_spmd(
        nc, in_maps, list(range(B)), trace=trace, **kwargs
    )
    out = np.stack(
        [res.results[b]["out"].reshape(4 * O4) for b in range(B)]
    ).astype(np.float32)
    return out, res


def kernel(x, w1, w2, w3, w4):
    out, _ = run(x, w1, w2, w3, w4)
    return out


# revision 4
# speedup vs baseline: 4.8090x; 4.8090x over previous
"""EquivariantDense kernel for Trainium2 (8 NeuronCores, data-parallel over batch).

Math: with K = 4096, N = 4K, shift = K, the reference computes
    out[b, i*O4 + o] = sum_j sum_k w_{j+1}[b, o, k] * x[b, ((i+j)%4)*K + k]
i.e. per batch, 4 weight matrices (1024, 4096) each hit the 4 chunks of x.

Device mapping (per core = one batch):
  - PE matmul out[m,n] = sum_p lhsT[p,m] * rhs[p,n] contracts over partitions,
    so weights are staged (on host) transposed to (k, o) layout.
  - stationary lhsT = x-chunk tile (128 k-part, 4 roll-columns)
  - moving rhs = W^T tile (128 k-part, 512 o)
  - accumulate all 128 k-blocks (4 j * 32 kb) into PSUM (4, 512) x 2 o-halves

Precision/bandwidth: weights are host-quantized to FP8_EXP3 (e3m4,
bias 3: 4-bit mantissa, max 15.5) after scaling by S=128 so the
~N(0, 0.0099^2) weights land mid-range (max|w*S| ~ 6.9; no inf
clipping). The inverse scale folds into x on host as an exact
power-of-2 exponent shift, so the device computes (S*w).(x/S) = w.x
with no rescale ops. Device-measured rel err: 1.14e-2 (gate 2e-2).
fp8 halves HBM traffic again vs bf16 (16 MiB/core, the memory-bound
floor at ~340 GB/s/core) and the moving-fp8 matmul runs the PE at
1 cycle/row, keeping PE (~47us) just under the DMA stream (~53us).
"""

import numpy as np
import ml_dtypes

import concourse.mybir as mybir
import concourse.tile as tile
from concourse import bacc, bass_utils

B = 8
O4 = 1024
K = 4096
N = 4 * K  # 16384
NBLK = N // 128  # 128 global k-blocks of 128
KB2 = 8  # k-blocks per DMA tile
NT = NBLK // KB2  # DMA tile count
RINGS = 1  # 1 = SP HWDGE only; 2 = whole tiles alternate SP/ACT

S = 128.0  # weight pre-scale (power of 2: x/S is exact)

BF16 = ml_dtypes.bfloat16
F8E3 = ml_dtypes.float8_e3m4

_nc_cache = None


def _build_program(repeat=1, loop=None):
    """repeat>1 unrolls the body; loop=K wraps the body in a hardware
    For_i loop (used for timing: device time scales with K at constant
    program size, so wall-clock dwarfs the axon transport noise)."""
    nc = bacc.Bacc()
    f32 = mybir.dt.float32
    bf16 = mybir.dt.bfloat16
    f8 = mybir.dt.float8e3
    xs_d = nc.dram_tensor("xstat", [128, NBLK * 4], bf16, kind="ExternalInput")
    wt_d = nc.dram_tensor("wt", [NT, 128, KB2 * O4], f8, kind="ExternalInput")
    out_d = nc.dram_tensor("out", [4, O4], f32, kind="ExternalOutput")

    with tile.TileContext(nc) as tc:
        with (
            tc.tile_pool(name="xp", bufs=1) as xp,
            tc.tile_pool(name="wp", bufs=4) as wp,
            tc.tile_pool(name="pp", bufs=2, space="PSUM") as pp,
            tc.tile_pool(name="op", bufs=2) as op,
        ):
            xs = xp.tile([128, NBLK * 4], bf16)
            # SWDGE keeps the SP HWDGE ring free for the weight stream.
            nc.gpsimd.dma_start(xs[:], xs_d[:])

            def body():
                ps0 = pp.tile([4, 512], f32, tag="ps0")
                # ps1 lives at PSUM base partition 32: the two per-block
                # matmuls then target different PE column groups (32-wide
                # sub-arrays), which the PE overlaps — measured ~26% faster
                # than both accumulating at partitions 0-3.
                ps1f = pp.tile([36, 512], f32, tag="ps1")
                ps1 = ps1f[32:36]
                for tidx, t in enumerate(range(NT)):
                    w_tile = wp.tile([128, KB2 * O4], f8, tag="w")
                    eng = nc.sync if (RINGS == 1 or t % 2 == 0) else nc.scalar
                    if tidx < NT - 1:
                        eng.dma_start(w_tile[:], wt_d[t])
                    else:
                        # split the last tile per k-block so the final
                        # matmuls chase the stream and the tail stays short
                        for kk in range(KB2 - 1):
                            eng.dma_start(
                                w_tile[:, kk * O4 : (kk + 1) * O4],
                                wt_d[t, :, kk * O4 : (kk + 1) * O4],
                            )
                        kk = KB2 - 1
                        eng.dma_start(
                            w_tile[:, kk * O4 : kk * O4 + 512],
                            wt_d[t, :, kk * O4 : kk * O4 + 512],
                        )
                        eng.dma_start(
                            w_tile[:, kk * O4 + 512 : (kk + 1) * O4],
                            wt_d[t, :, kk * O4 + 512 : (kk + 1) * O4],
                        )
                    for kb2 in range(KB2):
                        g = t * KB2 + kb2
                        lhsT = xs[:, g * 4 : (g + 1) * 4]
                        first = tidx == 0 and kb2 == 0
                        last = tidx == NT - 1 and kb2 == KB2 - 1
                        nc.tensor.matmul(
                            ps0[:],
                            lhsT,
                            w_tile[:, kb2 * O4 : kb2 * O4 + 512],
                            start=first,
                            stop=last,
                        )
                        nc.tensor.matmul(
                            ps1[:],
                            lhsT,
                            w_tile[:, kb2 * O4 + 512 : (kb2 + 1) * O4],
                            start=first,
                            stop=last,
                        )
                ot = op.tile([4, O4], f32, tag="ot")
                nc.vector.tensor_copy(ot[:, 0:512], ps0[:])
                nc.scalar.copy(ot[:, 512:O4], ps1[:])
                nc.sync.dma_start(out_d[:], ot[:])

            if loop is not None:
                with tc.For_i(0, loop, 1, hint_engines=(mybir.EngineType.PE,)):
                    body()
            else:
                for _rep in range(repeat):
                    body()
    nc.compile()
    return nc


def _get_program():
    global _nc_cache
    if _nc_cache is None:
        _nc_cache = _build_program()
    return _nc_cache


def prepare_inputs(x, w1, w2, w3, w4):
    """Host-side marshalling: shard over batch, quantize, transpose W to
    (k, o) tiles: Wh[b, t, p, kb2*O4 + o] = W[b, j, o, k] with
    k = (t*KB2 + kb2)*128 + p and j = (t*KB2 + kb2) // 32."""
    x = np.asarray(x, dtype=np.float32) * np.float32(1.0 / S)
    x = x.astype(BF16)
    W = np.stack(
        [
            (np.asarray(w, dtype=np.float32) * np.float32(S)).astype(F8E3)
            for w in (w1, w2, w3, w4)
        ],
        axis=1,
    )  # (B, 4, O4, K) fp8
    W6 = W.reshape(B, 4, O4, K // (KB2 * 128), KB2, 128)
    Wh = np.ascontiguousarray(W6.transpose(0, 1, 3, 5, 4, 2)).reshape(
        B, NT, 128, KB2 * O4
    )

    # x staging: xs[b, p, g*4 + c] = x[b, ((c + g//32) % 4)*K + (g%32)*128 + p]
    cols = np.arange(NBLK * 4)
    g = cols // 4
    c = cols % 4
    j = g // 32
    kb = g % 32
    src_base = ((c + j) % 4) * K + kb * 128  # (512,)
    xs = x[:, src_base[None, :] + np.arange(128)[:, None]]  # (B, 128, 512)
    xs = np.ascontiguousarray(xs)
    return xs, Wh


def run(x, w1, w2, w3, w4, trace=False, **kwargs):
    xs, Wh = prepare_inputs(x, w1, w2, w3, w4)
    nc = _get_program()
    in_maps = [{"xstat": xs[b], "wt": Wh[b]} for b in range(B)]
    res = bass_utils.run_bass_kernel_spmd(
        nc, in_maps, list(range(B)), trace=trace, **kwargs
    )
    out = np.stack(
        [res.results[b]["out"].reshape(4 * O4) for b in range(B)]
    ).astype(np.float32)
    return out, res


def kernel(x, w1, w2, w3, w4):
    try:
        out, _ = run(x, w1, w2, w3, w4)
    except Exception:
        # One retry: transient NRT/axon device errors (seen rarely as
        # NRT_EXEC_UNIT_UNRECOVERABLE) usually clear on a fresh attempt.
        import time as _time

        _time.sleep(5)
        out, _ = run(x, w1, w2, w3, w4)
    return out


# revision 7
# speedup vs baseline: 4.9185x; 1.0228x over previous
"""EquivariantDense kernel for Trainium2 (8 NeuronCores, data-parallel over batch).

Math: with K = 4096, N = 4K, shift = K, the reference computes
    out[b, i*O4 + o] = sum_j sum_k w_{j+1}[b, o, k] * x[b, ((i+j)%4)*K + k]
i.e. per batch, 4 weight matrices (1024, 4096) each hit the 4 chunks of x.

Device mapping (per core = one batch):
  - PE matmul out[m,n] = sum_p lhsT[p,m] * rhs[p,n] contracts over partitions,
    so weights are staged (on host) transposed to (k, o) layout.
  - stationary lhsT = x-chunk tile (128 k-part, 4 roll-columns)
  - moving rhs = W^T tile (128 k-part, 512 o)
  - accumulate all 128 k-blocks (4 j * 32 kb) into PSUM (4, 512) x 2 o-halves

Precision/bandwidth: weights are host-quantized to FP8_EXP3 (e3m4,
bias 3: 4-bit mantissa, max 15.5) after scaling by S=128 so the
~N(0, 0.0099^2) weights land mid-range (max|w*S| ~ 6.9; no inf
clipping). The inverse scale folds into x on host as an exact
power-of-2 exponent shift, so the device computes (S*w).(x/S) = w.x
with no rescale ops. Device-measured rel err: 1.14e-2 (gate 2e-2).
fp8 halves HBM traffic again vs bf16 (16 MiB/core, the memory-bound
floor at ~340 GB/s/core) and the moving-fp8 matmul runs the PE at
1 cycle/row, keeping PE (~47us) just under the DMA stream (~53us).
"""

import numpy as np
import ml_dtypes

import concourse.mybir as mybir
import concourse.tile as tile
from concourse import bacc, bass_utils

B = 8
O4 = 1024
K = 4096
N = 4 * K  # 16384
NBLK = N // 128  # 128 global k-blocks of 128
KB2 = 8  # k-blocks per DMA tile
NT = NBLK // KB2  # DMA tile count
RINGS = 1  # 1 = SP HWDGE only; 2 = whole tiles alternate SP/ACT
# PE column-group split of the o dim: accumulators at PSUM bases 0/32/64
CS_OFF = [0, 384, 768, 1024]  # o-chunk boundaries
NCS = len(CS_OFF) - 1
CS_W = [CS_OFF[i + 1] - CS_OFF[i] for i in range(NCS)]

S = 128.0  # weight pre-scale (power of 2: x/S is exact)

BF16 = ml_dtypes.bfloat16
F8E3 = ml_dtypes.float8_e3m4

_nc_cache = None


def _build_program(repeat=1, loop=None):
    """repeat>1 unrolls the body; loop=K wraps the body in a hardware
    For_i loop (used for timing: device time scales with K at constant
    program size, so wall-clock dwarfs the axon transport noise)."""
    nc = bacc.Bacc()
    f32 = mybir.dt.float32
    bf16 = mybir.dt.bfloat16
    f8 = mybir.dt.float8e3
    xs_d = nc.dram_tensor("xstat", [128, NBLK * 4], bf16, kind="ExternalInput")
    wt_d = nc.dram_tensor("wt", [NT, 128, KB2 * O4], f8, kind="ExternalInput")
    out_d = nc.dram_tensor("out", [4, O4], f32, kind="ExternalOutput")

    with tile.TileContext(nc) as tc:
        with (
            tc.tile_pool(name="xp", bufs=1) as xp,
            tc.tile_pool(name="wp", bufs=4) as wp,
            tc.tile_pool(name="pp", bufs=2, space="PSUM") as pp,
            tc.tile_pool(name="op", bufs=2) as op,
        ):
            xs = xp.tile([128, NBLK * 4], bf16)
            # SWDGE keeps the SP HWDGE ring free for the weight stream.
            nc.gpsimd.dma_start(xs[:], xs_d[:])

            def body():
                # Accumulators at PSUM base partitions 0/32/(64): each
                # per-block matmul then targets a different PE column group
                # (32-wide sub-array), which the PE overlaps — measured ~26%
                # faster than all accumulating at partitions 0-3. Base 96
                # (quadrant 3) is HW-broken, so at most 3 groups.
                pss = []
                for i in range(NCS):
                    psf = pp.tile(
                        [4 + 32 * i, CS_W[i]], f32, tag=f"ps{i}", name=f"psf{i}"
                    )
                    pss.append(psf[32 * i : 32 * i + 4])
                for tidx, t in enumerate(range(NT)):
                    w_tile = wp.tile([128, KB2 * O4], f8, tag="w")
                    eng = nc.sync if (RINGS == 1 or t % 2 == 0) else nc.scalar
                    if tidx < NT - 1:
                        eng.dma_start(w_tile[:], wt_d[t])
                    else:
                        # split the last tile per k-block so the final
                        # matmuls chase the stream and the tail stays short
                        for kk in range(KB2 - 1):
                            eng.dma_start(
                                w_tile[:, kk * O4 : (kk + 1) * O4],
                                wt_d[t, :, kk * O4 : (kk + 1) * O4],
                            )
                        kk = KB2 - 1
                        for i in range(NCS):
                            eng.dma_start(
                                w_tile[
                                    :, kk * O4 + CS_OFF[i] : kk * O4 + CS_OFF[i + 1]
                                ],
                                wt_d[t, :, kk * O4 + CS_OFF[i] : kk * O4 + CS_OFF[i + 1]],
                            )
                    for kb2 in range(KB2):
                        g = t * KB2 + kb2
                        lhsT = xs[:, g * 4 : (g + 1) * 4]
                        first = tidx == 0 and kb2 == 0
                        last = tidx == NT - 1 and kb2 == KB2 - 1
                        for i in range(NCS):
                            nc.tensor.matmul(
                                pss[i][:],
                                lhsT,
                                w_tile[
                                    :, kb2 * O4 + CS_OFF[i] : kb2 * O4 + CS_OFF[i + 1]
                                ],
                                start=first,
                                stop=last,
                            )
                for i in range(NCS):
                    oti = op.tile(
                        [4 + 32 * i, CS_W[i]], f32, tag=f"ot{i}", name=f"oti{i}"
                    )
                    dst = oti[32 * i : 32 * i + 4]
                    if i % 2 == 0:
                        nc.vector.tensor_copy(dst, pss[i][:])
                    else:
                        nc.scalar.copy(dst, pss[i][:])
                    nc.sync.dma_start(out_d[:, CS_OFF[i] : CS_OFF[i + 1]], dst)

            if loop is not None:
                with tc.For_i(0, loop, 1, hint_engines=(mybir.EngineType.PE,)):
                    body()
            else:
                for _rep in range(repeat):
                    body()
    nc.compile()
    return nc


def _get_program():
    global _nc_cache
    if _nc_cache is None:
        _nc_cache = _build_program()
    return _nc_cache


def prepare_inputs(x, w1, w2, w3, w4):
    """Host-side marshalling: shard over batch, quantize, transpose W to
    (k, o) tiles: Wh[b, t, p, kb2*O4 + o] = W[b, j, o, k] with
    k = (t*KB2 + kb2)*128 + p and j = (t*KB2 + kb2) // 32."""
    x = np.asarray(x, dtype=np.float32) * np.float32(1.0 / S)
    x = x.astype(BF16)
    W = np.stack(
        [
            (np.asarray(w, dtype=np.float32) * np.float32(S)).astype(F8E3)
            for w in (w1, w2, w3, w4)
        ],
        axis=1,
    )  # (B, 4, O4, K) fp8
    W6 = W.reshape(B, 4, O4, K // (KB2 * 128), KB2, 128)
    Wh = np.ascontiguousarray(W6.transpose(0, 1, 3, 5, 4, 2)).reshape(
        B, NT, 128, KB2 * O4
    )

    # x staging: xs[b, p, g*4 + c] = x[b, ((c + g//32) % 4)*K + (g%32)*128 + p]
    cols = np.arange(NBLK * 4)
    g = cols // 4
    c = cols % 4
    j = g // 32
    kb = g % 32
    src_base = ((c + j) % 4) * K + kb * 128  # (512,)
    xs = x[:, src_base[None, :] + np.arange(128)[:, None]]  # (B, 128, 512)
    xs = np.ascontiguousarray(xs)
    return xs, Wh


def run(x, w1, w2, w3, w4, trace=False, **kwargs):
    xs, Wh = prepare_inputs(x, w1, w2, w3, w4)
    nc = _get_program()
    in_maps = [{"xstat": xs[b], "wt": Wh[b]} for b in range(B)]
    res = bass_utils.run_bass_kernel_spmd(
        nc, in_maps, list(range(B)), trace=trace, **kwargs
    )
    out = np.stack(
        [res.results[b]["out"].reshape(4 * O4) for b in range(B)]
    ).astype(np.float32)
    return out, res


def kernel(x, w1, w2, w3, w4):
    try:
        out, _ = run(x, w1, w2, w3, w4)
    except Exception:
        # One retry: transient NRT/axon device errors (seen rarely as
        # NRT_EXEC_UNIT_UNRECOVERABLE) usually clear on a fresh attempt.
        import time as _time

        _time.sleep(5)
        out, _ = run(x, w1, w2, w3, w4)
    return out
